# revision 5
# baseline (speedup 1.0000x reference)
"""Quantized dense MLP kernel for 8 Trainium2 NeuronCores.

Problem: out = relu(inputs @ ((w_int8 - zero_point) * scale) + b)
  inputs [8192, 2048] f32, w_quantized [2048, 8192] int8,
  scale/zero_point f32 scalars, b [8192] f32 -> out [8192, 8192] f32.

Strategy:
- Data-parallel: shard rows of `inputs` across 8 cores (1024 rows each).
- Zero-point folding: w_int = w_int8 - zero_point (zero_point = -3.0) is a
  small integer, exactly representable in bf16. The scale and bias are
  applied on the ScalarEngine in f32: out = Relu(scale * acc + b).
- Everything in the matmul path is bf16. f32r weights were measured
  (trace) to saturate the PE weight-load port: LDWEIGHTS ~220 ns vs the
  213 ns N=512 matmul stream -> 278 ns/MM effective. bf16 weights get
  the compiler's fast-weight-load (64 cyc) and hide entirely: 216 ns/MM.
  Weights are exact in bf16; x rounding gives ~1.7e-3 rel err.
- On device each core computes outT[j, i] = sum_k w_int[k, j] * xT[k, i]
  (w tile stationary, xT moving), so the bias b_j is a natural
  per-partition activation bias. Output is written bf16 (relu output
  rounding ~2e-3) and the host casts back to f32 and transposes.
- Per core: x^T stays SBUF-resident; weights stream as G large 2D DMAs
  (the sync engine pays ~600 ns issue per DMA regardless of size, so few
  big transfers). x tiles are interleaved across BOTH hw queues (sync +
  act) behind the first j-tile's weights so the PE starts ~17 us in and
  x is fully resident ~10 us later; 8 PSUM banks keep 8 accumulation
  groups open so the PE keeps pace while x is still landing.
"""

import numpy as np
import ml_dtypes

import concourse.bass as bass
import concourse.mybir as mybir
import concourse.tile as tile
from concourse import bacc
from concourse.bass_utils import run_bass_kernel_spmd

BF16 = ml_dtypes.bfloat16

# Full problem dims (hardcoded per harness contract).
ROWS, D_IN, UNITS = 8192, 2048, 8192
N_CORES = 8
ROWS_C = ROWS // N_CORES  # rows per core

P = 128         # SBUF partitions
N_SLICE = 512   # moving free dim per matmul (one PSUM bank of f32)


def build_nc(scale: float, d_in: int = D_IN, units: int = UNITS,
             rows_c: int = ROWS_C):
    """Build + compile the per-core Bass program (SPMD, identical on all
    cores).

    DRAM inputs (per core):
      xt [KT, 128, rows_c]  bf16 : x-shard transposed, k-tiled
      w  [G, 128, JG*KT*128] bf16 : w_int, G groups of JG j-tiles;
            w[g, p, jtl*KT*128 + kt*128 + f] = w_int[kt*128+p, (g*JG+jtl)*128+f]
            so each group is one [128 x JG*KT*128] 2D DMA (16KB/partition).
      bt [128, JT]          f32  : bias, bt[p, jt] = b[jt*128 + p]
    DRAM output:
      o  [JT, 128, rows_c]  bf16 : outT tiles, o[jt, p, i] = outT[jt*128+p, i]
    """
    KT = d_in // P
    JT = units // P
    NS = rows_c // N_SLICE
    w_dt = mybir.dt.bfloat16
    x_dt = mybir.dt.bfloat16
    JG = min(8, JT)   # j-tiles per weight DMA group
    G = JT // JG
    WBUFS = 3

    nc = bacc.Bacc(None, target_bir_lowering=False)
    xt = nc.dram_tensor("xt", [KT, P, rows_c], x_dt, kind="ExternalInput")
    w = nc.dram_tensor("w", [G, P, JG * KT * P], w_dt, kind="ExternalInput")
    bt = nc.dram_tensor("bt", [P, JT], mybir.dt.float32, kind="ExternalInput")
    o = nc.dram_tensor("o", [JT, P, rows_c], mybir.dt.bfloat16,
                       kind="ExternalOutput")

    with tile.TileContext(nc) as tc:
        with (
            tc.tile_pool(name="xpool", bufs=1) as xpool,
            tc.tile_pool(name="bpool", bufs=1) as bpool,
            tc.tile_pool(name="wpool", bufs=WBUFS) as wpool,
            tc.tile_pool(name="opool", bufs=3) as opool,
            tc.tile_pool(name="pspool", bufs=8, space="PSUM") as pspool,
        ):
            # Prologue: the first j-tile's weights lead the SP HW queue;
            # x k-tiles are interleaved across BOTH queues right behind
            # (each queue sustains ~370 GB/s but they share the ~HBM port,
            # so the split mainly reduces head-of-line latency).
            wsbs = [wpool.tile([P, JG * KT * P], w_dt,
                               tag="wsb", name=f"wsb{g}") for g in range(G)]

            def w0_piece(eng, jtl):
                eng.dma_start(
                    out=wsbs[0][:, jtl * KT * P:(jtl + 1) * KT * P],
                    in_=w[0, :, jtl * KT * P:(jtl + 1) * KT * P],
                )

            # Prologue: the PE consumes one j-tile (32 MMs) per ~7 us but
            # needs every x k-tile to close an accumulation group, so the
            # queue heads interleave w g0 j-tiles with x k-tiles on BOTH
            # hw queues (jtl0 leads; jtl_i arrives well before ~7i us).
            w0_piece(nc.sync, 0)
            xsb = xpool.tile([P, KT * rows_c], x_dt)

            def x_piece(eng, kt):
                eng.dma_start(
                    out=xsb[:, kt * rows_c:(kt + 1) * rows_c], in_=xt[kt]
                )

            # SP queue: jtl0, x1, jtl1, x3, jtl2, x5, jtl3, x7, x9, ...
            # ACT queue: x0, x2, jtl4, x4, jtl5, x6, jtl6, x8, jtl7, ...
            x_piece(nc.scalar, 0)
            x_piece(nc.sync, 1)
            x_piece(nc.scalar, 2)
            w0_piece(nc.sync, 1)
            w0_piece(nc.scalar, 4)
            x_piece(nc.sync, 3)
            x_piece(nc.scalar, 4)
            w0_piece(nc.sync, 2)
            w0_piece(nc.scalar, 5)
            x_piece(nc.sync, 5)
            x_piece(nc.scalar, 6)
            w0_piece(nc.sync, 3)
            w0_piece(nc.scalar, 6)
            x_piece(nc.sync, 7)
            x_piece(nc.scalar, 8)
            w0_piece(nc.scalar, 7)
            for kt in range(9, KT):
                x_piece(nc.sync if kt % 2 else nc.scalar, kt)
            bsb = bpool.tile([P, JT], mybir.dt.float32)
            nc.scalar.dma_start(out=bsb[:, :], in_=bt[:, :])

            for g in range(G):
                wsb = wsbs[g]
                if g > 0:
                    # Prefetch on the Activation HW queue, queued behind the
                    # x tiles: keeps the early weight groups from stealing
                    # HBM bandwidth from the critical x stream, while the SP
                    # queue carries only w group 0 + output writes.
                    nc.scalar.dma_start(out=wsb[:, :], in_=w[g])
                for jtl in range(JG):
                    jt = g * JG + jtl
                    ob = opool.tile([P, rows_c], mybir.dt.bfloat16)
                    # kt-outer so consecutive matmuls reuse the stationary
                    # weight tile (one weight load serves all NS n-slices).
                    pss = [pspool.tile([P, N_SLICE], mybir.dt.float32,
                                       name="ps", tag="ps") for n in range(NS)]
                    for kt in range(KT):
                        wof = jtl * KT * P + kt * P
                        for n in range(NS):
                            nc.tensor.matmul(
                                pss[n][:, :],
                                wsb[:, wof:wof + P],
                                xsb[:, kt * rows_c + n * N_SLICE:
                                       kt * rows_c + (n + 1) * N_SLICE],
                                start=(kt == 0),
                                stop=(kt == KT - 1),
                            )
                    for n in range(NS):
                        nc.scalar.activation(
                            ob[:, n * N_SLICE:(n + 1) * N_SLICE],
                            pss[n][:, :],
                            mybir.ActivationFunctionType.Relu,
                            bias=bsb[:, jt:jt + 1],
                            scale=float(scale),
                        )
                        # Per-half output writes: the first half's DMA
                        # overlaps the second half's activation (tail).
                        nc.sync.dma_start(
                            out=o[jt, :, n * N_SLICE:(n + 1) * N_SLICE],
                            in_=ob[:, n * N_SLICE:(n + 1) * N_SLICE],
                        )

    nc.compile()
    return nc


def prep_w(w_int, d_in: int = None, units: int = None):
    """[d_in, units] -> [G, 128, JG*KT*128]; see build_nc docstring."""
    d_in = d_in or w_int.shape[0]
    units = units or w_int.shape[1]
    KT, JT = d_in // P, units // P
    JG = min(8, JT)
    G = JT // JG
    return np.ascontiguousarray(
        w_int.reshape(KT, P, G, JG, P)        # [kt, p, g, jtl, f]
             .transpose(2, 1, 3, 0, 4)        # [g, p, jtl, kt, f]
             .reshape(G, P, JG * KT * P)
    )


_NC_CACHE: dict = {}


def _get_nc(scale: float):
    key = round(float(scale), 12)
    if key not in _NC_CACHE:
        _NC_CACHE[key] = build_nc(float(scale))
    return _NC_CACHE[key]


def kernel(inputs, w_quantized, quantized_scale, zero_point, b):
    scale = float(np.asarray(quantized_scale))
    zp = float(np.asarray(zero_point))

    # Exact integer weights in bf16 (w - zp with zp = -3.0 stays a small
    # integer; bf16 represents integers up to 256 exactly).
    w_int = (np.asarray(w_quantized).astype(np.float32) - zp).astype(BF16)
    w_tiled = prep_w(w_int)

    bt = np.ascontiguousarray(
        np.asarray(b).astype(np.float32).reshape(UNITS // P, P).T
    )

    x_bf = np.asarray(inputs).astype(np.float32).astype(BF16)

    in_maps = []
    for c in range(N_CORES):
        shard = x_bf[c * ROWS_C:(c + 1) * ROWS_C, :]          # [1024, 2048]
        xt_c = np.ascontiguousarray(shard.T).reshape(D_IN // P, P, ROWS_C)
        in_maps.append({"xt": xt_c, "w": w_tiled, "bt": bt})

    nc = _get_nc(scale)
    results = run_bass_kernel_spmd(nc, in_maps, core_ids=list(range(N_CORES)))
    global _LAST_RESULTS
    _LAST_RESULTS = results

    out = np.empty((ROWS, UNITS), dtype=np.float32)
    for c in range(N_CORES):
        outT = results.results[c]["o"].astype(np.float32).reshape(UNITS, ROWS_C)
        out[c * ROWS_C:(c + 1) * ROWS_C, :] = outT.T
    return out


# revision 6
# speedup vs baseline: 1.0067x; 1.0067x over previous
"""Quantized dense MLP kernel for 8 Trainium2 NeuronCores.

Problem: out = relu(inputs @ ((w_int8 - zero_point) * scale) + b)
  inputs [8192, 2048] f32, w_quantized [2048, 8192] int8,
  scale/zero_point f32 scalars, b [8192] f32 -> out [8192, 8192] f32.

Strategy:
- Data-parallel: shard rows of `inputs` across 8 cores (1024 rows each).
- Zero-point folding: w_int = w_int8 - zero_point (zero_point = -3.0) is a
  small integer, exactly representable in bf16. The scale and bias are
  applied on the ScalarEngine in f32: out = Relu(scale * acc + b).
- Everything in the matmul path is bf16. f32r weights were measured
  (trace) to saturate the PE weight-load port: LDWEIGHTS ~220 ns vs the
  213 ns N=512 matmul stream -> 278 ns/MM effective. bf16 weights get
  the compiler's fast-weight-load (64 cyc) and hide entirely: 216 ns/MM.
  Weights are exact in bf16; x rounding gives ~1.7e-3 rel err.
- On device each core computes outT[j, i] = sum_k w_int[k, j] * xT[k, i]
  (w tile stationary, xT moving), so the bias b_j is a natural
  per-partition activation bias. Output is written bf16 (relu output
  rounding ~2e-3) and the host casts back to f32 and transposes.
- Per core: x^T stays SBUF-resident; weights stream as G large 2D DMAs
  (the sync engine pays ~600 ns issue per DMA regardless of size, so few
  big transfers). x tiles are interleaved across BOTH hw queues (sync +
  act) behind the first j-tile's weights so the PE starts ~17 us in and
  x is fully resident ~10 us later; 8 PSUM banks keep 8 accumulation
  groups open so the PE keeps pace while x is still landing.
"""

import numpy as np
import ml_dtypes

import concourse.bass as bass
import concourse.mybir as mybir
import concourse.tile as tile
from concourse import bacc
from concourse.bass_utils import run_bass_kernel_spmd

BF16 = ml_dtypes.bfloat16

# Full problem dims (hardcoded per harness contract).
ROWS, D_IN, UNITS = 8192, 2048, 8192
N_CORES = 8
ROWS_C = ROWS // N_CORES  # rows per core

P = 128         # SBUF partitions
N_SLICE = 512   # moving free dim per matmul (one PSUM bank of f32)


def build_nc(scale: float, d_in: int = D_IN, units: int = UNITS,
             rows_c: int = ROWS_C):
    """Build + compile the per-core Bass program (SPMD, identical on all
    cores).

    DRAM inputs (per core):
      xt [KT, 128, rows_c]  bf16 : x-shard transposed, k-tiled
      w  [G, 128, JG*KT*128] bf16 : w_int, G groups of JG j-tiles;
            w[g, p, jtl*KT*128 + kt*128 + f] = w_int[kt*128+p, (g*JG+jtl)*128+f]
            so each group is one [128 x JG*KT*128] 2D DMA (16KB/partition).
      bt [128, JT]          f32  : bias, bt[p, jt] = b[jt*128 + p]
    DRAM output:
      o  [JT, 128, rows_c]  bf16 : outT tiles, o[jt, p, i] = outT[jt*128+p, i]
    """
    KT = d_in // P
    JT = units // P
    NS = rows_c // N_SLICE
    w_dt = mybir.dt.bfloat16
    x_dt = mybir.dt.bfloat16
    JG = min(8, JT)   # j-tiles per weight DMA group
    G = JT // JG
    WBUFS = 3

    nc = bacc.Bacc(None, target_bir_lowering=False)
    xt = nc.dram_tensor("xt", [KT, P, rows_c], x_dt, kind="ExternalInput")
    w = nc.dram_tensor("w", [G, P, JG * KT * P], w_dt, kind="ExternalInput")
    bt = nc.dram_tensor("bt", [P, JT], mybir.dt.float32, kind="ExternalInput")
    o = nc.dram_tensor("o", [JT, P, rows_c], mybir.dt.bfloat16,
                       kind="ExternalOutput")

    with tile.TileContext(nc) as tc:
        with (
            tc.tile_pool(name="xpool", bufs=1) as xpool,
            tc.tile_pool(name="bpool", bufs=1) as bpool,
            tc.tile_pool(name="wpool", bufs=WBUFS) as wpool,
            tc.tile_pool(name="opool", bufs=3) as opool,
            tc.tile_pool(name="pspool", bufs=8, space="PSUM") as pspool,
        ):
            # Prologue: the first j-tile's weights lead the SP HW queue;
            # x k-tiles are interleaved across BOTH queues right behind
            # (each queue sustains ~370 GB/s but they share the ~HBM port,
            # so the split mainly reduces head-of-line latency).
            wsbs = [wpool.tile([P, JG * KT * P], w_dt,
                               tag="wsb", name=f"wsb{g}") for g in range(G)]

            def w0_piece(eng, jtl):
                eng.dma_start(
                    out=wsbs[0][:, jtl * KT * P:(jtl + 1) * KT * P],
                    in_=w[0, :, jtl * KT * P:(jtl + 1) * KT * P],
                )

            # Prologue: the PE consumes one j-tile (32 MMs) per ~7 us but
            # needs every x k-tile to close an accumulation group, so the
            # queue heads interleave w g0 j-tiles with x k-tiles on BOTH
            # hw queues (jtl0 leads; jtl_i arrives well before ~7i us).
            w0_piece(nc.sync, 0)
            xsb = xpool.tile([P, KT * rows_c], x_dt)

            def x_piece(eng, kt):
                eng.dma_start(
                    out=xsb[:, kt * rows_c:(kt + 1) * rows_c], in_=xt[kt]
                )

            # SP queue: jtl0, x1, jtl1, x3, jtl2, x5, jtl3, x7, x9, ...
            # ACT queue: x0, x2, jtl4, x4, jtl5, x6, jtl6, x8, jtl7, ...
            x_piece(nc.scalar, 0)
            x_piece(nc.sync, 1)
            x_piece(nc.scalar, 2)
            w0_piece(nc.sync, 1)
            w0_piece(nc.scalar, 4)
            x_piece(nc.sync, 3)
            x_piece(nc.scalar, 4)
            w0_piece(nc.sync, 2)
            w0_piece(nc.scalar, 5)
            x_piece(nc.sync, 5)
            x_piece(nc.scalar, 6)
            w0_piece(nc.sync, 3)
            w0_piece(nc.scalar, 6)
            x_piece(nc.sync, 7)
            x_piece(nc.scalar, 8)
            w0_piece(nc.scalar, 7)
            for kt in range(9, KT):
                x_piece(nc.sync if kt % 2 else nc.scalar, kt)
            bsb = bpool.tile([P, JT], mybir.dt.float32)
            nc.scalar.dma_start(out=bsb[:, :], in_=bt[:, :])

            def mm(pss_jt, wsb, jtl, kt, n):
                wof = jtl * KT * P + kt * P
                nc.tensor.matmul(
                    pss_jt[n][:, :],
                    wsb[:, wof:wof + P],
                    xsb[:, kt * rows_c + n * N_SLICE:
                           kt * rows_c + (n + 1) * N_SLICE],
                    start=(kt == 0),
                    stop=(kt == KT - 1),
                )

            def act_and_store(pss_jt, jt, chunks=1):
                # chunks>1 splits each 512-col activation + store so the
                # final output chain after the last matmul is short.
                ob = opool.tile([P, rows_c], mybir.dt.bfloat16)
                cw = N_SLICE // chunks
                for n in range(NS):
                    for c in range(chunks):
                        lo = n * N_SLICE + c * cw
                        nc.scalar.activation(
                            ob[:, lo:lo + cw],
                            pss_jt[n][:, c * cw:(c + 1) * cw],
                            mybir.ActivationFunctionType.Relu,
                            bias=bsb[:, jt:jt + 1],
                            scale=float(scale),
                        )
                        nc.sync.dma_start(
                            out=o[jt, :, lo:lo + cw], in_=ob[:, lo:lo + cw]
                        )

            def new_pss():
                return [pspool.tile([P, N_SLICE], mybir.dt.float32,
                                    name="ps", tag="ps") for _ in range(NS)]

            # Warmup block: the first WJ j-tiles run kt-MAJOR across all
            # 8 PSUM banks, so every x k-tile that lands unlocks 8 matmuls
            # (~1.7 us of PE work per ~0.7 us of DMA) and the PE stays
            # busy while x is still streaming in.
            WJ = 8 // NS
            warm_pss = [new_pss() for _ in range(WJ)]
            for kt in range(KT):
                for jtl in range(WJ):
                    for n in range(NS):
                        mm(warm_pss[jtl], wsbs[0], jtl, kt, n)
            for jtl in range(WJ):
                act_and_store(warm_pss[jtl], jtl)

            for g in range(G):
                wsb = wsbs[g]
                if g > 0:
                    # Prefetch on the Activation HW queue, queued behind the
                    # x tiles: keeps the early weight groups from stealing
                    # HBM bandwidth from the critical x stream, while the SP
                    # queue carries only w group 0 + output writes.
                    nc.scalar.dma_start(out=wsb[:, :], in_=w[g])
                for jtl in range(WJ if g == 0 else 0, JG):
                    jt = g * JG + jtl
                    pss = new_pss()
                    for kt in range(KT):
                        for n in range(NS):
                            mm(pss, wsb, jtl, kt, n)
                    last = (g == G - 1) and (jtl == JG - 1)
                    act_and_store(pss, jt, chunks=4 if last else 1)

    nc.compile()
    return nc


def prep_w(w_int, d_in: int = None, units: int = None):
    """[d_in, units] -> [G, 128, JG*KT*128]; see build_nc docstring."""
    d_in = d_in or w_int.shape[0]
    units = units or w_int.shape[1]
    KT, JT = d_in // P, units // P
    JG = min(8, JT)
    G = JT // JG
    return np.ascontiguousarray(
        w_int.reshape(KT, P, G, JG, P)        # [kt, p, g, jtl, f]
             .transpose(2, 1, 3, 0, 4)        # [g, p, jtl, kt, f]
             .reshape(G, P, JG * KT * P)
    )


_NC_CACHE: dict = {}


def _get_nc(scale: float):
    key = round(float(scale), 12)
    if key not in _NC_CACHE:
        _NC_CACHE[key] = build_nc(float(scale))
    return _NC_CACHE[key]


def kernel(inputs, w_quantized, quantized_scale, zero_point, b):
    scale = float(np.asarray(quantized_scale))
    zp = float(np.asarray(zero_point))

    # Exact integer weights in bf16 (w - zp with zp = -3.0 stays a small
    # integer; bf16 represents integers up to 256 exactly).
    w_int = (np.asarray(w_quantized).astype(np.float32) - zp).astype(BF16)
    w_tiled = prep_w(w_int)

    bt = np.ascontiguousarray(
        np.asarray(b).astype(np.float32).reshape(UNITS // P, P).T
    )

    x_bf = np.asarray(inputs).astype(np.float32).astype(BF16)

    in_maps = []
    for c in range(N_CORES):
        shard = x_bf[c * ROWS_C:(c + 1) * ROWS_C, :]          # [1024, 2048]
        xt_c = np.ascontiguousarray(shard.T).reshape(D_IN // P, P, ROWS_C)
        in_maps.append({"xt": xt_c, "w": w_tiled, "bt": bt})

    nc = _get_nc(scale)
    results = run_bass_kernel_spmd(nc, in_maps, core_ids=list(range(N_CORES)))
    global _LAST_RESULTS
    _LAST_RESULTS = results

    out = np.empty((ROWS, UNITS), dtype=np.float32)
    for c in range(N_CORES):
        outT = results.results[c]["o"].astype(np.float32).reshape(UNITS, ROWS_C)
        out[c * ROWS_C:(c + 1) * ROWS_C, :] = outT.T
    return out


# revision 8
# speedup vs baseline: 1.0119x; 1.0052x over previous
"""Quantized dense MLP kernel for 8 Trainium2 NeuronCores.

Problem: out = relu(inputs @ ((w_int8 - zero_point) * scale) + b)
  inputs [8192, 2048] f32, w_quantized [2048, 8192] int8,
  scale/zero_point f32 scalars, b [8192] f32 -> out [8192, 8192] f32.

Strategy:
- Data-parallel: shard rows of `inputs` across 8 cores (1024 rows each).
- Zero-point folding: w_int = w_int8 - zero_point (zero_point = -3.0) is a
  small integer, exactly representable in bf16. The scale and bias are
  applied on the ScalarEngine in f32: out = Relu(scale * acc + b).
- Everything in the matmul path is bf16. f32r weights were measured
  (trace) to saturate the PE weight-load port: LDWEIGHTS ~220 ns vs the
  213 ns N=512 matmul stream -> 278 ns/MM effective. bf16 weights get
  the compiler's fast-weight-load (64 cyc) and hide entirely: 216 ns/MM.
  Weights are exact in bf16; x rounding gives ~1.7e-3 rel err.
- On device each core computes outT[j, i] = sum_k w_int[k, j] * xT[k, i]
  (w tile stationary, xT moving), so the bias b_j is a natural
  per-partition activation bias. Output is written bf16 (relu output
  rounding ~2e-3) and the host casts back to f32 and transposes.
- Per core: x^T stays SBUF-resident; weights stream as G large 2D DMAs
  (the sync engine pays ~600 ns issue per DMA regardless of size, so few
  big transfers). x tiles are interleaved across BOTH hw queues (sync +
  act) behind the first j-tile's weights so the PE starts ~17 us in and
  x is fully resident ~10 us later; 8 PSUM banks keep 8 accumulation
  groups open so the PE keeps pace while x is still landing.
"""

import numpy as np
import ml_dtypes

import concourse.bass as bass
import concourse.mybir as mybir
import concourse.tile as tile
from concourse import bacc
from concourse.bass_utils import run_bass_kernel_spmd

BF16 = ml_dtypes.bfloat16

# Full problem dims (hardcoded per harness contract).
ROWS, D_IN, UNITS = 8192, 2048, 8192
N_CORES = 8
ROWS_C = ROWS // N_CORES  # rows per core

P = 128         # SBUF partitions
N_SLICE = 512   # moving free dim per matmul (one PSUM bank of f32)


def build_nc(scale: float, d_in: int = D_IN, units: int = UNITS,
             rows_c: int = ROWS_C):
    """Build + compile the per-core Bass program (SPMD, identical on all
    cores).

    DRAM inputs (per core):
      xt [KT, 128, rows_c]  bf16 : x-shard transposed, k-tiled
      w  [G, 128, JG*KT*128] bf16 : w_int, G groups of JG j-tiles;
            w[g, p, jtl*KT*128 + kt*128 + f] = w_int[kt*128+p, (g*JG+jtl)*128+f]
            so each group is one [128 x JG*KT*128] 2D DMA (16KB/partition).
      bt [128, JT]          f32  : bias, bt[p, jt] = b[jt*128 + p]
    DRAM output:
      o  [JT, 128, rows_c]  bf16 : outT tiles, o[jt, p, i] = outT[jt*128+p, i]
    """
    KT = d_in // P
    JT = units // P
    NS = rows_c // N_SLICE
    w_dt = mybir.dt.bfloat16
    x_dt = mybir.dt.bfloat16
    JG = min(8, JT)   # j-tiles per weight DMA group
    G = JT // JG
    WBUFS = 3

    nc = bacc.Bacc(None, target_bir_lowering=False)
    xt = nc.dram_tensor("xt", [KT, P, rows_c], x_dt, kind="ExternalInput")
    w = nc.dram_tensor("w", [G, P, JG * KT * P], w_dt, kind="ExternalInput")
    bt = nc.dram_tensor("bt", [P, JT], mybir.dt.float32, kind="ExternalInput")
    o = nc.dram_tensor("o", [JT, P, rows_c], mybir.dt.bfloat16,
                       kind="ExternalOutput")

    with tile.TileContext(nc) as tc:
        with (
            tc.tile_pool(name="xpool", bufs=1) as xpool,
            tc.tile_pool(name="bpool", bufs=1) as bpool,
            tc.tile_pool(name="wpool", bufs=WBUFS) as wpool,
            tc.tile_pool(name="opool", bufs=3) as opool,
            tc.tile_pool(name="pspool", bufs=8, space="PSUM") as pspool,
        ):
            # Prologue: the first j-tile's weights lead the SP HW queue;
            # x k-tiles are interleaved across BOTH queues right behind
            # (each queue sustains ~370 GB/s but they share the ~HBM port,
            # so the split mainly reduces head-of-line latency).
            wsbs = [wpool.tile([P, JG * KT * P], w_dt,
                               tag="wsb", name=f"wsb{g}") for g in range(G)]

            def w0_piece(eng, jtl):
                eng.dma_start(
                    out=wsbs[0][:, jtl * KT * P:(jtl + 1) * KT * P],
                    in_=w[0, :, jtl * KT * P:(jtl + 1) * KT * P],
                )

            # Prologue: the PE consumes one j-tile (32 MMs) per ~7 us but
            # needs every x k-tile to close an accumulation group, so the
            # queue heads interleave w g0 j-tiles with x k-tiles on BOTH
            # hw queues (jtl0 leads; jtl_i arrives well before ~7i us).
            w0_piece(nc.sync, 0)
            xsb = xpool.tile([P, KT * rows_c], x_dt)

            def x_piece(eng, kt):
                eng.dma_start(
                    out=xsb[:, kt * rows_c:(kt + 1) * rows_c], in_=xt[kt]
                )

            # SP queue: jtl0, x1, jtl1, x3, jtl2, x5, jtl3, x7, x9, ...
            # ACT queue: x0, x2, jtl4, x4, jtl5, x6, jtl6, x8, jtl7, ...
            x_piece(nc.scalar, 0)
            x_piece(nc.sync, 1)
            x_piece(nc.scalar, 2)
            w0_piece(nc.sync, 1)
            w0_piece(nc.scalar, 4)
            x_piece(nc.sync, 3)
            x_piece(nc.scalar, 4)
            w0_piece(nc.sync, 2)
            w0_piece(nc.scalar, 5)
            x_piece(nc.sync, 5)
            x_piece(nc.scalar, 6)
            w0_piece(nc.sync, 3)
            w0_piece(nc.scalar, 6)
            x_piece(nc.sync, 7)
            x_piece(nc.scalar, 8)
            w0_piece(nc.scalar, 7)
            for kt in range(9, KT):
                x_piece(nc.sync if kt % 2 else nc.scalar, kt)
            bsb = bpool.tile([P, JT], mybir.dt.float32)
            nc.scalar.dma_start(out=bsb[:, :], in_=bt[:, :])

            def mm(pss_jt, wsb, jtl, kt, n):
                wof = jtl * KT * P + kt * P
                nc.tensor.matmul(
                    pss_jt[n][:, :],
                    wsb[:, wof:wof + P],
                    xsb[:, kt * rows_c + n * N_SLICE:
                           kt * rows_c + (n + 1) * N_SLICE],
                    start=(kt == 0),
                    stop=(kt == KT - 1),
                )

            def act_and_store(pss_jt, jt, chunks=1):
                # chunks>1 splits each 512-col activation + store so the
                # final output chain after the last matmul is short.
                ob = opool.tile([P, rows_c], mybir.dt.bfloat16)
                cw = N_SLICE // chunks
                for n in range(NS):
                    for c in range(chunks):
                        lo = n * N_SLICE + c * cw
                        nc.scalar.activation(
                            ob[:, lo:lo + cw],
                            pss_jt[n][:, c * cw:(c + 1) * cw],
                            mybir.ActivationFunctionType.Relu,
                            bias=bsb[:, jt:jt + 1],
                            scale=float(scale),
                        )
                        nc.sync.dma_start(
                            out=o[jt, :, lo:lo + cw], in_=ob[:, lo:lo + cw]
                        )

            def new_pss():
                return [pspool.tile([P, N_SLICE], mybir.dt.float32,
                                    name="ps", tag="ps") for _ in range(NS)]

            # Dummy matmuls on a memset tile: the PE's HAM clock-gate only
            # un-throttles (1.2 -> 2.4 GHz) after ~3.4 us of sustained
            # activity, and the engine preamble ends ~1.5 us before the
            # first weights land. Burning that window on garbage matmuls
            # (write-only PSUM group, freed by the pool's WAW rotation)
            # makes the real matmuls run warm from the start.
            dsb = xpool.tile([P, N_SLICE], x_dt, name="dsb", tag="dsb")
            nc.gpsimd.memset(dsb[:, :], 0)
            dps = pspool.tile([P, N_SLICE], mybir.dt.float32,
                              name="ps", tag="ps")
            for i in range(8):
                nc.tensor.matmul(
                    dps[:, :], dsb[:, :P], dsb[:, :],
                    start=(i == 0), stop=(i == 7),
                )

            # Warmup block: the first WJ j-tiles run kt-MAJOR across all
            # 8 PSUM banks, so every x k-tile that lands unlocks up to 8
            # matmuls (~1.7 us of PE work per ~0.7 us of DMA) and the PE
            # stays busy while x is still streaming in. The block is
            # skewed ("wavefront"): j-tile j joins at step j, matching the
            # order its weights land on the queues.
            WJ = 8 // NS
            warm_pss = [new_pss() for _ in range(WJ)]
            for step in range(KT + WJ - 1):
                for jtl in range(WJ):
                    kt = step - jtl
                    if 0 <= kt < KT:
                        for n in range(NS):
                            mm(warm_pss[jtl], wsbs[0], jtl, kt, n)
            for jtl in range(WJ):
                act_and_store(warm_pss[jtl], jtl)

            for g in range(G):
                wsb = wsbs[g]
                if g > 0:
                    # Prefetch on the Activation HW queue, queued behind the
                    # x tiles: keeps the early weight groups from stealing
                    # HBM bandwidth from the critical x stream, while the SP
                    # queue carries only w group 0 + output writes.
                    nc.scalar.dma_start(out=wsb[:, :], in_=w[g])
                for jtl in range(WJ if g == 0 else 0, JG):
                    jt = g * JG + jtl
                    pss = new_pss()
                    for kt in range(KT):
                        for n in range(NS):
                            mm(pss, wsb, jtl, kt, n)
                    act_and_store(pss, jt)

    nc.compile()
    return nc


def prep_w(w_int, d_in: int = None, units: int = None):
    """[d_in, units] -> [G, 128, JG*KT*128]; see build_nc docstring."""
    d_in = d_in or w_int.shape[0]
    units = units or w_int.shape[1]
    KT, JT = d_in // P, units // P
    JG = min(8, JT)
    G = JT // JG
    return np.ascontiguousarray(
        w_int.reshape(KT, P, G, JG, P)        # [kt, p, g, jtl, f]
             .transpose(2, 1, 3, 0, 4)        # [g, p, jtl, kt, f]
             .reshape(G, P, JG * KT * P)
    )


_NC_CACHE: dict = {}


def _get_nc(scale: float):
    key = round(float(scale), 12)
    if key not in _NC_CACHE:
        _NC_CACHE[key] = build_nc(float(scale))
    return _NC_CACHE[key]


def kernel(inputs, w_quantized, quantized_scale, zero_point, b):
    scale = float(np.asarray(quantized_scale))
    zp = float(np.asarray(zero_point))

    # Exact integer weights in bf16 (w - zp with zp = -3.0 stays a small
    # integer; bf16 represents integers up to 256 exactly).
    w_int = (np.asarray(w_quantized).astype(np.float32) - zp).astype(BF16)
    w_tiled = prep_w(w_int)

    bt = np.ascontiguousarray(
        np.asarray(b).astype(np.float32).reshape(UNITS // P, P).T
    )

    x_bf = np.asarray(inputs).astype(np.float32).astype(BF16)

    in_maps = []
    for c in range(N_CORES):
        shard = x_bf[c * ROWS_C:(c + 1) * ROWS_C, :]          # [1024, 2048]
        xt_c = np.ascontiguousarray(shard.T).reshape(D_IN // P, P, ROWS_C)
        in_maps.append({"xt": xt_c, "w": w_tiled, "bt": bt})

    nc = _get_nc(scale)
    results = run_bass_kernel_spmd(nc, in_maps, core_ids=list(range(N_CORES)))
    global _LAST_RESULTS
    _LAST_RESULTS = results

    out = np.empty((ROWS, UNITS), dtype=np.float32)
    for c in range(N_CORES):
        outT = results.results[c]["o"].astype(np.float32).reshape(UNITS, ROWS_C)
        out[c * ROWS_C:(c + 1) * ROWS_C, :] = outT.T
    return out
